# revision 38
# baseline (speedup 1.0000x reference)
"""MoE routing kernel (2 experts, D=128 -> H=512 -> O=2) for 8 Trainium2 cores.

Strategy: expert-sorted sharding. The routing decision (a 128-dim dot vs a
threshold) is computed host-side as part of choosing the data distribution;
samples are stable-partitioned by expert, padded so every core receives the
identical layout (kb0 expert-0 blocks followed by kb1 expert-1 blocks of 512
samples), and uploaded pre-transposed in bf16. Each core then runs a pure
dense single-expert MLP per block — 8 back-to-back 512-row bf16 matmuls:

  per 512-sample block (expert e fixed at compile time):
    DMA xT tile [128d, 512b] bf16 (batched 4 blocks/transfer)
    PE  layer-1: 4 matmuls (w1 j-tiles stationary, xT moving) -> z PSUM
    ACT/DVE: relu(z + b1) -> h SBUF bf16   (two fused [128,1024] ops)
    PE  layer-2: 4 accumulating matmuls -> out PSUM; the w2 stationaries are
        zero-padded to 128 columns so every matmul has the same shape and
        LDWEIGHTS always hides behind the previous matmul's stream
    ACT/DVE (2:1): + b2, copy [2,512] to SBUF, DMA out per block pair

Emission is software-pipelined (layer-1 of block n before layer-2 of block
n-1) so the PE never waits on the relu engines, and warmup matmuls ramp the
PE to its top p-state while the first DMAs are in flight.  The host gathers
per-core [2, n] outputs and scatters rows back through the inverse
permutation.
"""

import numpy as np
import ml_dtypes

import concourse.bacc as bacc
import concourse.mybir as mybir
import concourse.tile as tile
from concourse.bass_utils import run_bass_kernel_spmd

F32 = mybir.dt.float32
BF16 = mybir.dt.bfloat16
BF16_NP = ml_dtypes.bfloat16

N_CORES = 8
D = 128
H = 512
E = 2
O = 2
NJ = H // 128         # 4 hidden k-tiles of 128 per expert
BLK = 512             # samples per block


def _build_program(nb: int, kb0: int):
    """Per-core program: nb blocks of 512; first kb0 blocks use expert 0."""
    nc = bacc.Bacc(
        "TRN2",
        target_bir_lowering=False,
        debug=False,
        enable_asserts=False,
        num_devices=1,
    )

    n_shard = nb * BLK
    WCOL = H + NJ * 128  # per-expert packed weights (w1t | w2r zero-padded to 128 cols/j)
    xt = nc.dram_tensor("xt", [D, n_shard], BF16, kind="ExternalInput").ap()
    # whead = weights of the first-used expert (whead0 = its first j-tile,
    # tiny, so block 0's first matmul starts as early as possible), wtail =
    # the other expert's
    whead0 = nc.dram_tensor("whead0", [D, 128], BF16, kind="ExternalInput").ap()
    whead1 = nc.dram_tensor("whead1", [D, H - 128], BF16, kind="ExternalInput").ap()
    whead2 = nc.dram_tensor("whead2", [D, WCOL - H], BF16, kind="ExternalInput").ap()
    wtail = nc.dram_tensor("wtail", [D, WCOL], BF16, kind="ExternalInput").ap()
    cf32 = nc.dram_tensor("cf32", [D, E * NJ + E], F32, kind="ExternalInput").ap()
    out = nc.dram_tensor("out", [O, n_shard], F32, kind="ExternalOutput").ap()

    with tile.TileContext(nc) as tc:
        _body(tc, nb, kb0, xt, whead0, whead1, whead2, wtail, cf32, out)

    nc.compile()
    return nc


def _body(tc, nb, kb0, xt, whead0, whead1, whead2, wtail, cf32, out):
    nc = tc.nc
    Relu = mybir.ActivationFunctionType.Relu
    Alu = mybir.AluOpType
    WCOL = H + NJ * 128
    e_first = 0 if kb0 > 0 else 1

    with (
        tc.tile_pool(name="consts", bufs=1) as cpool,
        tc.tile_pool(name="xs", bufs=4) as x_pool,
        tc.tile_pool(name="h", bufs=3) as h_pool,
        tc.tile_pool(name="os", bufs=3) as o_pool,
        tc.tile_pool(name="zp", bufs=3, space="PSUM") as zp_pool,
        tc.tile_pool(name="op", bufs=2, space="PSUM") as op_pool,
    ):
        # PE warmup: dummy matmuls ramp the tensor engine to its top
        # p-state while the input DMAs are still in flight
        scr = cpool.tile([D, 128 + BLK], BF16)
        nc.gpsimd.memset(scr[:], 0.0)
        zpw = op_pool.tile([D, BLK], F32, name="op")
        for _ in range(18):
            nc.tensor.matmul(
                zpw[:],
                lhsT=scr[:, 0:128],
                rhs=scr[:, 128 : 128 + BLK],
                start=True,
                stop=True,
            )

        # const DMAs issued from the ACT queue, in parallel with the first
        # x DMA on the Sync queue; the first-needed expert's weights first,
        # with its very first j-tile as a separate tiny transfer
        wh_sb = cpool.tile([D, WCOL], BF16)
        nc.scalar.dma_start(wh_sb[:, 0:128], whead0)
        nc.scalar.dma_start(wh_sb[:, 128:H], whead1)
        cf_sb = cpool.tile([D, E * NJ + E], F32)
        nc.scalar.dma_start(cf_sb[:], cf32)
        nc.scalar.dma_start(wh_sb[:, H:WCOL], whead2)
        wt_sb = cpool.tile([D, WCOL], BF16)
        nc.scalar.dma_start(wt_sb[:], wtail)
        wsb = [wh_sb, wt_sb] if e_first == 0 else [wt_sb, wh_sb]
        w1t_of = lambda e: wsb[e][:, 0:H]
        w2r_of = lambda e: wsb[e][:, H : H + NJ * 128]
        b1c_sb = cf_sb[:, 0 : E * NJ]
        b2c_sb = cf_sb[0:O, E * NJ : E * NJ + E]

        XB = 4  # x-in DMA batch (blocks)
        OB = 2  # out DMA batch (blocks)

        hs = [None] * nb
        ops = [None] * nb
        osbp = {}
        xq = None
        xq_base = 0

        def emit_l1(bi):
            nonlocal xq, xq_base
            e = 0 if bi < kb0 else 1
            # small first batches so early blocks start as soon as possible
            if bi in (0, 1, 3) or (bi >= 7 and (bi - 7) % XB == 0):
                t = {0: 1, 1: 2}.get(bi) or min(XB, nb - bi)
                t = min(t, nb - bi)
                xq = x_pool.tile([D, t, BLK], BF16, name="xq")
                xq_base = bi
                nc.sync.dma_start(
                    xq.rearrange("p t b -> p (t b)"),
                    xt[:, bi * BLK : (bi + t) * BLK],
                )
            h = h_pool.tile([D, NJ, BLK], BF16, name="h")
            hs[bi] = h
            for half in range(2):
                zp = zp_pool.tile([D, 2, BLK], F32, name="zp")
                for k in range(2):
                    j = half * 2 + k
                    nc.tensor.matmul(
                        zp[:, k, :],
                        lhsT=w1t_of(e)[:, j * 128 : (j + 1) * 128],
                        rhs=xq[:, bi - xq_base, :],
                        start=True,
                        stop=True,
                    )
                # relu(z + b1) -> h bf16; ACT for half 0, DVE for half 1
                j0 = half * 2
                if half == 0:
                    nc.scalar.activation(
                        h[:, j0 : j0 + 2, :],
                        zp[:],
                        Relu,
                        bias=b1c_sb[:, e * NJ + j0 : e * NJ + j0 + 1],
                        scale=1.0,
                    )
                else:
                    nc.vector.tensor_scalar(
                        out=h[:, j0 : j0 + 2, :],
                        in0=zp[:],
                        scalar1=b1c_sb[:, e * NJ + j0 : e * NJ + j0 + 1],
                        scalar2=0.0,
                        op0=Alu.add,
                        op1=Alu.max,
                    )

        def emit_l2(bi):
            e = 0 if bi < kb0 else 1
            h = hs[bi]
            op = op_pool.tile([D, BLK], F32, name="op")
            ops[bi] = op
            for j in range(NJ):
                nc.tensor.matmul(
                    op[:],
                    lhsT=w2r_of(e)[:, j * 128 : (j + 1) * 128],
                    rhs=h[:, j, :],
                    start=(j == 0),
                    stop=(j == NJ - 1),
                )

        def emit_out(bi):
            # +b2, PSUM->SBUF (whole op, alternating engine); DMA per pair
            e = 0 if bi < kb0 else 1
            m, t = divmod(bi, OB)
            tb = min(OB, nb - m * OB)
            if t == 0:
                osbp[m] = o_pool.tile([O, tb, BLK], F32, name="osb")
            osb = osbp[m]
            if bi % 3 < 2:
                nc.scalar.activation(
                    osb[:, t, :],
                    ops[bi][0:O, :],
                    mybir.ActivationFunctionType.Identity,
                    bias=b2c_sb[:, e : e + 1],
                    scale=1.0,
                )
            else:
                nc.vector.tensor_scalar(
                    out=osb[:, t, :],
                    in0=ops[bi][0:O, :],
                    scalar1=b2c_sb[:, e : e + 1],
                    scalar2=None,
                    op0=Alu.add,
                )
            ops[bi] = None
            if t == tb - 1:
                nc.sync.dma_start(
                    out[:, m * OB * BLK : (m * OB + tb) * BLK],
                    osb.rearrange("o t b -> o (t b)"),
                )

        # software-pipelined emission: PE runs L1(n) before L2(n-1) so it
        # never waits on the relu engines
        for bi in range(nb):
            emit_l1(bi)
            if bi >= 1:
                emit_l2(bi - 1)
                emit_out(bi - 1)
        emit_l2(nb - 1)
        emit_out(nb - 1)


_PROG_CACHE = {}


def _get_program(nb, kb0):
    key = (nb, kb0)
    if key not in _PROG_CACHE:
        _PROG_CACHE[key] = _build_program(nb, kb0)
    return _PROG_CACHE[key]


def kernel(x, w1, b1, w2, b2, prototypes, _trace=False):
    x = np.ascontiguousarray(np.asarray(x, np.float32))
    w1 = np.asarray(w1, np.float32)
    b1 = np.asarray(b1, np.float32)
    w2 = np.asarray(w2, np.float32)
    b2 = np.asarray(b2, np.float32)
    p = np.asarray(prototypes, np.float64)
    btot = x.shape[0]

    # host routing (argmin over squared distance == threshold test on the
    # projection onto p1-p0); expert 0 wins ties like argmin does
    rvec = p[1] - p[0]
    thr = (p[1] @ p[1] - p[0] @ p[0]) / 2.0
    q = x.astype(np.float64) @ rvec
    t1 = q > thr
    idx0 = np.flatnonzero(~t1)
    idx1 = np.flatnonzero(t1)
    n0, n1 = idx0.size, idx1.size

    # pad each expert's block count to a multiple of 8 so all cores get the
    # same (kb0, kb1) layout and run one SPMD program
    kb0 = -(-(-(-n0 // BLK)) // N_CORES)
    kb1 = -(-(-(-n1 // BLK)) // N_CORES)
    nb = kb0 + kb1
    ns = nb * BLK  # samples per core (with padding)

    xe = np.zeros((N_CORES * ns, D), np.float32)
    e0x = x[idx0]
    e1x = x[idx1]
    c0, c1 = kb0 * BLK, kb1 * BLK
    for c in range(N_CORES):
        s0 = c * c0
        z0 = min(max(n0 - s0, 0), c0)
        if z0:
            xe[c * ns : c * ns + z0] = e0x[s0 : s0 + z0]
        s1 = c * c1
        z1 = min(max(n1 - s1, 0), c1)
        if z1:
            xe[c * ns + c0 : c * ns + c0 + z1] = e1x[s1 : s1 + z1]
    xtb = np.ascontiguousarray(xe.T.astype(BF16_NP))  # [128, 8*ns]

    # per-expert packed weights [w1t | w2r(128-wide)] bf16
    wpk = []
    b1c = np.zeros((D, E * NJ), np.float32)
    for e in range(E):
        w2r = np.zeros((D, NJ * 128), np.float32)
        for j in range(NJ):
            for o in range(O):
                w2r[:, j * 128 + o] = w2[e, o, j * 128 : (j + 1) * 128]
            b1c[:, e * NJ + j] = b1[e, j * 128 : (j + 1) * 128]
        wpk.append(
            np.concatenate([w1[e].T, w2r], axis=1).astype(BF16_NP)
        )
    cf32 = np.zeros((D, E * NJ + E), np.float32)
    cf32[:, : E * NJ] = b1c
    cf32[:O, E * NJ :] = b2.T  # cf32[o, E*NJ+e] = b2[e, o]

    e_first = 0 if kb0 > 0 else 1
    nc = _get_program(nb, kb0)
    consts = dict(
        whead0=np.ascontiguousarray(wpk[e_first][:, :128]),
        whead1=np.ascontiguousarray(wpk[e_first][:, 128:H]),
        whead2=np.ascontiguousarray(wpk[e_first][:, H:]),
        wtail=wpk[1 - e_first],
        cf32=cf32,
    )
    in_maps = []
    for c in range(N_CORES):
        m = dict(consts)
        m["xt"] = np.ascontiguousarray(xtb[:, c * ns : (c + 1) * ns])
        in_maps.append(m)

    res = run_bass_kernel_spmd(
        nc, in_maps, core_ids=list(range(N_CORES)), trace=_trace
    )

    # gather: per-core [2, ns] -> rows, drop padding, inverse permutation
    oute = np.stack(
        [res.results[c]["out"].T for c in range(N_CORES)]
    )  # [8, ns, 2]
    full = np.empty((btot, O), np.float32)
    if n0:
        full[idx0] = oute[:, :c0, :].reshape(N_CORES * c0, O)[:n0]
    if n1:
        full[idx1] = oute[:, c0:, :].reshape(N_CORES * c1, O)[:n1]
    if _trace:
        return full, res
    return full


# revision 39
# speedup vs baseline: 1.0077x; 1.0077x over previous
"""MoE routing kernel (2 experts, D=128 -> H=512 -> O=2) for 8 Trainium2 cores.

Strategy: expert-sorted sharding. The routing decision (a 128-dim dot vs a
threshold) is computed host-side as part of choosing the data distribution;
samples are stable-partitioned by expert, padded so every core receives the
identical layout (kb0 expert-0 blocks followed by kb1 expert-1 blocks of 512
samples), and uploaded pre-transposed in bf16. Each core then runs a pure
dense single-expert MLP per block — 8 back-to-back 512-row bf16 matmuls:

  per 512-sample block (expert e fixed at compile time):
    DMA xT tile [128d, 512b] bf16 (batched 4 blocks/transfer)
    PE  layer-1: 4 matmuls (w1 j-tiles stationary, xT moving) -> z PSUM
    ACT/DVE: relu(z + b1) -> h SBUF bf16   (two fused [128,1024] ops)
    PE  layer-2: 4 accumulating matmuls -> out PSUM; the w2 stationaries are
        zero-padded to 128 columns so every matmul has the same shape and
        LDWEIGHTS always hides behind the previous matmul's stream
    ACT/DVE (2:1): + b2, copy [2,512] to SBUF, DMA out per block pair

Emission is software-pipelined (layer-1 of block n before layer-2 of block
n-1) so the PE never waits on the relu engines, and warmup matmuls ramp the
PE to its top p-state while the first DMAs are in flight.  The host gathers
per-core [2, n] outputs and scatters rows back through the inverse
permutation.
"""

import numpy as np
import ml_dtypes

import concourse.bacc as bacc
import concourse.mybir as mybir
import concourse.tile as tile
from concourse.bass_utils import run_bass_kernel_spmd

F32 = mybir.dt.float32
BF16 = mybir.dt.bfloat16
BF16_NP = ml_dtypes.bfloat16

N_CORES = 8
D = 128
H = 512
E = 2
O = 2
NJ = H // 128         # 4 hidden k-tiles of 128 per expert
BLK = 512             # samples per block


def _build_program(nb: int, kb0: int):
    """Per-core program: nb blocks of 512; first kb0 blocks use expert 0."""
    nc = bacc.Bacc(
        "TRN2",
        target_bir_lowering=False,
        debug=False,
        enable_asserts=False,
        num_devices=1,
    )

    n_shard = nb * BLK
    WCOL = H + NJ * 128  # per-expert packed weights (w1t | w2r zero-padded to 128 cols/j)
    xt = nc.dram_tensor("xt", [D, n_shard], BF16, kind="ExternalInput").ap()
    # whead = weights of the first-used expert (whead0 = its first j-tile,
    # tiny, so block 0's first matmul starts as early as possible), wtail =
    # the other expert's
    whead0 = nc.dram_tensor("whead0", [D, 128], BF16, kind="ExternalInput").ap()
    whead1 = nc.dram_tensor("whead1", [D, H - 128], BF16, kind="ExternalInput").ap()
    whead2 = nc.dram_tensor("whead2", [D, WCOL - H], BF16, kind="ExternalInput").ap()
    wtail = nc.dram_tensor("wtail", [D, WCOL], BF16, kind="ExternalInput").ap()
    cf32 = nc.dram_tensor("cf32", [D, E * NJ + E], F32, kind="ExternalInput").ap()
    out = nc.dram_tensor("out", [O, n_shard], F32, kind="ExternalOutput").ap()

    with tile.TileContext(nc) as tc:
        _body(tc, nb, kb0, xt, whead0, whead1, whead2, wtail, cf32, out)

    nc.compile()
    return nc


def _body(tc, nb, kb0, xt, whead0, whead1, whead2, wtail, cf32, out):
    nc = tc.nc
    Relu = mybir.ActivationFunctionType.Relu
    Alu = mybir.AluOpType
    WCOL = H + NJ * 128
    e_first = 0 if kb0 > 0 else 1

    with (
        tc.tile_pool(name="consts", bufs=1) as cpool,
        tc.tile_pool(name="xs", bufs=4) as x_pool,
        tc.tile_pool(name="h", bufs=3) as h_pool,
        tc.tile_pool(name="os", bufs=3) as o_pool,
        tc.tile_pool(name="zp", bufs=3, space="PSUM") as zp_pool,
        tc.tile_pool(name="op", bufs=2, space="PSUM") as op_pool,
    ):
        # PE warmup: dummy matmuls ramp the tensor engine to its top
        # p-state while the input DMAs are still in flight
        scr = cpool.tile([D, 128 + BLK], BF16)
        nc.gpsimd.memset(scr[:], 0.0)
        zpw = op_pool.tile([D, BLK], F32, name="op")
        for _ in range(18):
            nc.tensor.matmul(
                zpw[:],
                lhsT=scr[:, 0:128],
                rhs=scr[:, 128 : 128 + BLK],
                start=True,
                stop=True,
            )

        # const DMAs issued from the ACT queue, in parallel with the first
        # x DMA on the Sync queue; the first-needed expert's weights first,
        # with its very first j-tile as a separate tiny transfer
        wh_sb = cpool.tile([D, WCOL], BF16)
        nc.scalar.dma_start(wh_sb[:, 0:128], whead0)
        nc.scalar.dma_start(wh_sb[:, 128:H], whead1)
        cf_sb = cpool.tile([D, E * NJ + E], F32)
        nc.scalar.dma_start(cf_sb[:], cf32)
        nc.scalar.dma_start(wh_sb[:, H:WCOL], whead2)
        wt_sb = cpool.tile([D, WCOL], BF16)
        nc.scalar.dma_start(wt_sb[:], wtail)
        wsb = [wh_sb, wt_sb] if e_first == 0 else [wt_sb, wh_sb]
        w1t_of = lambda e: wsb[e][:, 0:H]
        w2r_of = lambda e: wsb[e][:, H : H + NJ * 128]
        b1c_sb = cf_sb[:, 0 : E * NJ]
        b2c_sb = cf_sb[0:O, E * NJ : E * NJ + E]

        XB = 4  # x-in DMA batch (blocks)
        OB = 2  # out DMA batch (blocks)

        hs = [None] * nb
        ops = [None] * nb
        osbp = {}
        xq = None
        xq_base = 0

        def emit_l1(bi):
            nonlocal xq, xq_base
            e = 0 if bi < kb0 else 1
            # small first batches so early blocks start as soon as possible
            if bi in (0, 1, 2, 4) or (bi >= 8 and (bi - 8) % XB == 0):
                t = {0: 1, 1: 1, 2: 2}.get(bi) or min(XB, nb - bi)
                t = min(t, nb - bi)
                xq = x_pool.tile([D, t, BLK], BF16, name="xq")
                xq_base = bi
                nc.sync.dma_start(
                    xq.rearrange("p t b -> p (t b)"),
                    xt[:, bi * BLK : (bi + t) * BLK],
                )
            h = h_pool.tile([D, NJ, BLK], BF16, name="h")
            hs[bi] = h
            for half in range(2):
                zp = zp_pool.tile([D, 2, BLK], F32, name="zp")
                for k in range(2):
                    j = half * 2 + k
                    nc.tensor.matmul(
                        zp[:, k, :],
                        lhsT=w1t_of(e)[:, j * 128 : (j + 1) * 128],
                        rhs=xq[:, bi - xq_base, :],
                        start=True,
                        stop=True,
                    )
                # relu(z + b1) -> h bf16; ACT for half 0, DVE for half 1
                j0 = half * 2
                if half == 0:
                    nc.scalar.activation(
                        h[:, j0 : j0 + 2, :],
                        zp[:],
                        Relu,
                        bias=b1c_sb[:, e * NJ + j0 : e * NJ + j0 + 1],
                        scale=1.0,
                    )
                else:
                    nc.vector.tensor_scalar(
                        out=h[:, j0 : j0 + 2, :],
                        in0=zp[:],
                        scalar1=b1c_sb[:, e * NJ + j0 : e * NJ + j0 + 1],
                        scalar2=0.0,
                        op0=Alu.add,
                        op1=Alu.max,
                    )

        def emit_l2(bi):
            e = 0 if bi < kb0 else 1
            h = hs[bi]
            op = op_pool.tile([D, BLK], F32, name="op")
            ops[bi] = op
            for j in range(NJ):
                nc.tensor.matmul(
                    op[:],
                    lhsT=w2r_of(e)[:, j * 128 : (j + 1) * 128],
                    rhs=h[:, j, :],
                    start=(j == 0),
                    stop=(j == NJ - 1),
                )

        def emit_out(bi):
            # +b2, PSUM->SBUF (whole op, alternating engine); DMA per pair
            e = 0 if bi < kb0 else 1
            m, t = divmod(bi, OB)
            tb = min(OB, nb - m * OB)
            if t == 0:
                osbp[m] = o_pool.tile([O, tb, BLK], F32, name="osb")
            osb = osbp[m]
            if bi % 3 < 2:
                nc.scalar.activation(
                    osb[:, t, :],
                    ops[bi][0:O, :],
                    mybir.ActivationFunctionType.Identity,
                    bias=b2c_sb[:, e : e + 1],
                    scale=1.0,
                )
            else:
                nc.vector.tensor_scalar(
                    out=osb[:, t, :],
                    in0=ops[bi][0:O, :],
                    scalar1=b2c_sb[:, e : e + 1],
                    scalar2=None,
                    op0=Alu.add,
                )
            ops[bi] = None
            if t == tb - 1:
                nc.sync.dma_start(
                    out[:, m * OB * BLK : (m * OB + tb) * BLK],
                    osb.rearrange("o t b -> o (t b)"),
                )

        # software-pipelined emission: PE runs L1(n) before L2(n-1) so it
        # never waits on the relu engines
        for bi in range(nb):
            emit_l1(bi)
            if bi >= 1:
                emit_l2(bi - 1)
                emit_out(bi - 1)
        emit_l2(nb - 1)
        emit_out(nb - 1)


_PROG_CACHE = {}


def _get_program(nb, kb0):
    key = (nb, kb0)
    if key not in _PROG_CACHE:
        _PROG_CACHE[key] = _build_program(nb, kb0)
    return _PROG_CACHE[key]


def kernel(x, w1, b1, w2, b2, prototypes, _trace=False):
    x = np.ascontiguousarray(np.asarray(x, np.float32))
    w1 = np.asarray(w1, np.float32)
    b1 = np.asarray(b1, np.float32)
    w2 = np.asarray(w2, np.float32)
    b2 = np.asarray(b2, np.float32)
    p = np.asarray(prototypes, np.float64)
    btot = x.shape[0]

    # host routing (argmin over squared distance == threshold test on the
    # projection onto p1-p0); expert 0 wins ties like argmin does
    rvec = p[1] - p[0]
    thr = (p[1] @ p[1] - p[0] @ p[0]) / 2.0
    q = x.astype(np.float64) @ rvec
    t1 = q > thr
    idx0 = np.flatnonzero(~t1)
    idx1 = np.flatnonzero(t1)
    n0, n1 = idx0.size, idx1.size

    # pad each expert's block count to a multiple of 8 so all cores get the
    # same (kb0, kb1) layout and run one SPMD program
    kb0 = -(-(-(-n0 // BLK)) // N_CORES)
    kb1 = -(-(-(-n1 // BLK)) // N_CORES)
    nb = kb0 + kb1
    ns = nb * BLK  # samples per core (with padding)

    xe = np.zeros((N_CORES * ns, D), np.float32)
    e0x = x[idx0]
    e1x = x[idx1]
    c0, c1 = kb0 * BLK, kb1 * BLK
    for c in range(N_CORES):
        s0 = c * c0
        z0 = min(max(n0 - s0, 0), c0)
        if z0:
            xe[c * ns : c * ns + z0] = e0x[s0 : s0 + z0]
        s1 = c * c1
        z1 = min(max(n1 - s1, 0), c1)
        if z1:
            xe[c * ns + c0 : c * ns + c0 + z1] = e1x[s1 : s1 + z1]
    xtb = np.ascontiguousarray(xe.T.astype(BF16_NP))  # [128, 8*ns]

    # per-expert packed weights [w1t | w2r(128-wide)] bf16
    wpk = []
    b1c = np.zeros((D, E * NJ), np.float32)
    for e in range(E):
        w2r = np.zeros((D, NJ * 128), np.float32)
        for j in range(NJ):
            for o in range(O):
                w2r[:, j * 128 + o] = w2[e, o, j * 128 : (j + 1) * 128]
            b1c[:, e * NJ + j] = b1[e, j * 128 : (j + 1) * 128]
        wpk.append(
            np.concatenate([w1[e].T, w2r], axis=1).astype(BF16_NP)
        )
    cf32 = np.zeros((D, E * NJ + E), np.float32)
    cf32[:, : E * NJ] = b1c
    cf32[:O, E * NJ :] = b2.T  # cf32[o, E*NJ+e] = b2[e, o]

    e_first = 0 if kb0 > 0 else 1
    nc = _get_program(nb, kb0)
    consts = dict(
        whead0=np.ascontiguousarray(wpk[e_first][:, :128]),
        whead1=np.ascontiguousarray(wpk[e_first][:, 128:H]),
        whead2=np.ascontiguousarray(wpk[e_first][:, H:]),
        wtail=wpk[1 - e_first],
        cf32=cf32,
    )
    in_maps = []
    for c in range(N_CORES):
        m = dict(consts)
        m["xt"] = np.ascontiguousarray(xtb[:, c * ns : (c + 1) * ns])
        in_maps.append(m)

    res = run_bass_kernel_spmd(
        nc, in_maps, core_ids=list(range(N_CORES)), trace=_trace
    )

    # gather: per-core [2, ns] -> rows, drop padding, inverse permutation
    oute = np.stack(
        [res.results[c]["out"].T for c in range(N_CORES)]
    )  # [8, ns, 2]
    full = np.empty((btot, O), np.float32)
    if n0:
        full[idx0] = oute[:, :c0, :].reshape(N_CORES * c0, O)[:n0]
    if n1:
        full[idx1] = oute[:, c0:, :].reshape(N_CORES * c1, O)[:n1]
    if _trace:
        return full, res
    return full
